# revision 12
# baseline (speedup 1.0000x reference)
"""MultiHeadAttention forward on 8 Trainium2 NeuronCores (Bass/Tile).

Problem: B=2, N=2048, C=1024, H=16, D=64, fp32.
  q/k/v = x @ W* + b*; scores = q k^T / sqrt(D); w = softmax(scores);
  out = (w v) @ Wo + bo.

Sharding (no collectives): core c handles batch b = c//4 and query block
qb = c%4 (512 queries). Each core computes K/V for its whole batch
(4x duplicated within a batch group) and attention + output projection for
its 512 queries.

Key trick: inputs are passed per-core as x[b].T cyclically ROTATED by the
query offset (softmax is permutation-invariant over keys), so a single SPMD
program always works on queries at columns 0:512.

Layout: everything runs transposed (feature-major). Matmuls use float32r
(fp32 with 11-bit mantissa, 4x faster than fp32 on the PE); PSUM
accumulation is full fp32.

Per-core pipeline:
  A1: QT[c,512q]   = Wq^T @ xT[:, :512]        (+bq via DVE bias-add)
  A2: KT[c,2048k]  = Wk^T @ xT                 (+bk) -> SBUF resident
  A3: V[2048k,c]   = xT^T @ Wv (+bv via ones-trick matmul) -> DRAM staging
  B:  per head h: scoresT = KT_h^T QT_h; e = exp(0.125*scoresT) (ACT);
      avT[65,512] = [V_h | 1]^T @ e (accumulate over key tiles; row 64 =
      softmax denominators); normalize via reciprocal + PE broadcast.
  C:  yT = Wo^T @ attT (+bo), DMA out as [1024, 512] (host transposes).
"""
import os
import numpy as np

B, N, C, H, D = 2, 2048, 1024, 16, 64
NCORES = 8
QBLK = 512          # queries per core
GROUPS = 4          # query blocks per batch
KT_TILES = C // 128  # 8
RT_TILES = N // 128  # 16

_CACHE = {}


def _build_nc(nrep: int = 1, kdtype: str = "f32r", small_out: bool = False):
    import concourse.bacc as bacc
    import concourse.mybir as mybir
    import concourse.tile as tile
    from concourse.bass import ts, ds

    f32 = mybir.dt.float32
    fr = mybir.dt.float32r if kdtype == "f32r" else mybir.dt.float32

    nc = bacc.Bacc("TRN2", target_bir_lowering=False, debug=False)

    # ---- I/O ----
    xT = nc.dram_tensor("xT", [C, N], fr, kind="ExternalInput")
    wq = nc.dram_tensor("wq", [C, C], fr, kind="ExternalInput")
    wk = nc.dram_tensor("wk", [C, C], fr, kind="ExternalInput")
    wv = nc.dram_tensor("wv", [C, C], fr, kind="ExternalInput")
    wo = nc.dram_tensor("wo", [C, C], fr, kind="ExternalInput")
    bq = nc.dram_tensor("bq", [128, 8], f32, kind="ExternalInput")
    bk = nc.dram_tensor("bk", [128, 8], f32, kind="ExternalInput")
    bo = nc.dram_tensor("bo", [128, 8], f32, kind="ExternalInput")
    bv = nc.dram_tensor("bv", [1, C], fr, kind="ExternalInput")
    outT = nc.dram_tensor("outT", [128 if small_out else C, QBLK], f32,
                          kind="ExternalOutput")

    vd = nc.dram_tensor("vd", [N, C], fr)  # V staging (natural layout)

    EXPF = mybir.ActivationFunctionType.Exp
    IDENT = mybir.ActivationFunctionType.Identity

    with tile.TileContext(nc) as tc:
        with tc.tile_pool(name="persist", bufs=1) as pp, \
             tc.tile_pool(name="wpool", bufs=8) as wp:
            # constants / persistent tiles
            ones = pp.tile([128, 128], fr, name="ones")
            nc.vector.memset(ones[:].bitcast(mybir.dt.uint32), 0x3F800000)
            bq_sb = pp.tile([128, 8], f32, name="bq_sb")
            bk_sb = pp.tile([128, 8], f32, name="bk_sb")
            bo_sb = pp.tile([128, 8], f32, name="bo_sb")
            bv_sb = pp.tile([1, C], fr, name="bv_sb")
            nc.sync.dma_start(out=bq_sb[:], in_=bq[:])
            nc.sync.dma_start(out=bk_sb[:], in_=bk[:])
            nc.sync.dma_start(out=bo_sb[:], in_=bo[:])
            nc.sync.dma_start(out=bv_sb[:], in_=bv[:])

            qt_sb = [pp.tile([128, QBLK], fr, name=f"qt{m}")
                     for m in range(KT_TILES)]
            kt_sb = [pp.tile([128, N], fr, name=f"kt{m}")
                     for m in range(KT_TILES)]

            for rep in range(nrep):
                _emit_once(nc, tc, tile, mybir, ts, ds, fr, f32, EXPF, IDENT,
                           xT, wq, wk, wv, wo, bq_sb, bk_sb, bo_sb, bv_sb,
                           ones, qt_sb, kt_sb, vd, outT,
                           wp, rep, small_out)
    nc.compile()
    return nc


def _emit_once(nc, tc, tile, mybir, ts, ds, fr, f32, EXPF, IDENT,
               xT, wq, wk, wv, wo, bq_sb, bk_sb, bo_sb, bv_sb,
               ones, qt_sb, kt_sb, vd, outT, wp, rep, small_out=False):
    # ---------- Phase A: projections ----------
    with tc.tile_pool(name="xtp", bufs=8) as xp, \
         tc.tile_pool(name="vstage", bufs=3) as vsp, \
         tc.tile_pool(name="accA", bufs=1, space="PSUM") as accp:
        # A3 (first): V = xT^T @ Wv -> vd DRAM; bv added via DVE
        # against a PE-broadcast [128, C] bv tile. Interleave wv/xT loads
        # so rt=0's k-loop can start as soon as (wv_0, xt_0) land.
        xt_sb, wv_sb = [], []
        for k in range(KT_TILES):
            w = wp.tile([128, C], fr, name="w", tag="w")
            nc.sync.dma_start(out=w[:], in_=wv[ts(k, 128), :])
            wv_sb.append(w)
            xt = xp.tile([128, N], fr, name="xt", tag="xt")
            nc.sync.dma_start(out=xt[:], in_=xT[ts(k, 128), :])
            xt_sb.append(xt)
        bvb = xp.tile([128, C], f32, name="bvb", bufs=1)
        for j in range(2):
            bacc = accp.tile([128, 512], f32, name="bacc", tag="acck2",
                             bufs=1)
            nc.tensor.matmul(bacc[:], ones[0:1, 0:128],
                             bv_sb[0:1, ds(j * 512, 512)],
                             start=True, stop=True)
            nc.vector.tensor_copy(bvb[:, ds(j * 512, 512)], bacc[:])
        for rt in range(RT_TILES):
            accs = [accp.tile([128, 512], f32, name="acc", tag=f"acck{j}",
                              bufs=1)
                    for j in range(2)]
            for k in range(KT_TILES):
                for j in range(2):
                    nc.tensor.matmul(accs[j][:], xt_sb[k][:, ts(rt, 128)],
                                     wv_sb[k][:, ds(j * 512, 512)],
                                     start=(k == 0),
                                     stop=(k == KT_TILES - 1))
            for j in range(2):
                vst = vsp.tile([128, 512], fr, name="vst", tag="vst")
                nc.vector.tensor_add(vst[:], accs[j][:],
                                     bvb[:, ds(j * 512, 512)])
                nc.sync.dma_start(out=vd[ts(rt, 128), ds(j * 512, 512)],
                                  in_=vst[:])

        # A1: QT = Wq^T @ xT[:, 0:512]  (+bq)
        wq_sb = []
        for k in range(KT_TILES):
            w = wp.tile([128, C], fr, name="w", tag="w")
            nc.sync.dma_start(out=w[:], in_=wq[ts(k, 128), :])
            wq_sb.append(w)
        for m in range(KT_TILES):
            acc = accp.tile([128, QBLK], f32, name="acc", tag="acc", bufs=2)
            for k in range(KT_TILES):
                nc.tensor.matmul(acc[:], wq_sb[k][:, ts(m, 128)],
                                 xt_sb[k][:, 0:QBLK],
                                 start=(k == 0), stop=(k == KT_TILES - 1))
            nc.vector.tensor_scalar_add(qt_sb[m][:], acc[:],
                                        bq_sb[:, m:m + 1])

        # A2: KT = Wk^T @ xT  (+bk), kept in SBUF
        wk_sb = []
        for k in range(KT_TILES):
            w = wp.tile([128, C], fr, name="w", tag="w")
            nc.sync.dma_start(out=w[:], in_=wk[ts(k, 128), :])
            wk_sb.append(w)
        for m in range(KT_TILES):
            accs = [accp.tile([128, 512], f32, name="acc", tag=f"acck{j}",
                              bufs=1)
                    for j in range(4)]
            for k in range(KT_TILES):
                for j in range(4):
                    nc.tensor.matmul(accs[j][:], wk_sb[k][:, ts(m, 128)],
                                     xt_sb[k][:, ds(j * 512, 512)],
                                     start=(k == 0),
                                     stop=(k == KT_TILES - 1))
            for j in range(4):
                nc.vector.tensor_scalar_add(kt_sb[m][:, ds(j * 512, 512)],
                                            accs[j][:], bk_sb[:, m:m + 1])
    # ---------- Phase B: attention per head ----------
    vd_r = vd.rearrange("(t p) c -> p t c", p=128)
    with tc.tile_pool(name="attp", bufs=1) as ap:
      att_sb = [ap.tile([128, QBLK], fr, name=f"att{m}")
                for m in range(KT_TILES)]
      with tc.tile_pool(name="bwork", bufs=2) as bw, \
           tc.tile_pool(name="etp", bufs=3) as etp, \
           tc.tile_pool(name="scp", bufs=2, space="PSUM") as scp, \
           tc.tile_pool(name="avp", bufs=3, space="PSUM") as avp:
        for h in range(H):
            tI, pO = h // 2, (h % 2) * 64
            vh = bw.tile([128, RT_TILES, 65], fr, name="vh", tag="vh")
            nc.sync.dma_start(out=vh[:, :, 0:64],
                              in_=vd_r[:, :, ds(h * 64, 64)])
            nc.vector.memset(vh[:, :, 64].bitcast(mybir.dt.uint32),
                             0x3F800000)

            av = avp.tile([65, 512], f32, name="av", tag="avbc")
            for g in range(RT_TILES // 2):
                sc = scp.tile([128, 1024], f32, name="sc", tag="sc")
                for q in range(2):
                    kt = 2 * g + q
                    nc.tensor.matmul(
                        sc[:, ds(q * 512, 512)],
                        kt_sb[tI][pO:pO + 64, ts(kt, 128)],
                        qt_sb[tI][pO:pO + 64, :],
                        start=True, stop=True)
                et = etp.tile([128, 1024], fr, name="et", tag="et")
                nc.scalar.activation(et[:], sc[:], EXPF, bias=0.0,
                                     scale=0.125)
                for q in range(2):
                    kt = 2 * g + q
                    nc.tensor.matmul(av[:], vh[:, kt, :],
                                     et[:, ds(q * 512, 512)],
                                     start=(kt == 0),
                                     stop=(kt == RT_TILES - 1))
            rs = bw.tile([65, 512], fr, name="rs", tag="rs")
            with nc.allow_low_precision("f32r softmax denom"):
                nc.vector.reciprocal(rs[64:65, :], av[64:65, :])
            bc = avp.tile([64, 512], f32, name="bc", tag="avbc")
            nc.tensor.matmul(bc[:], ones[64:65, 0:64], rs[64:65, :],
                             start=True, stop=True)
            bcs = bw.tile([64, 512], f32, name="bcs", tag="bcs")
            nc.vector.tensor_copy(bcs[:], bc[:])
            if pO == 0:
                nc.vector.tensor_mul(att_sb[tI][0:64, :], av[0:64, :],
                                     bcs[:])
            else:
                tmp = bw.tile([64, 512], fr, name="tmpn", tag="tmpn")
                nc.vector.tensor_mul(tmp[:], av[0:64, :], bcs[:])
                nc.sync.dma_start(out=att_sb[tI][64:128, :], in_=tmp[:])

      # ---------- Phase C: output projection ----------
      with tc.tile_pool(name="ostage", bufs=3) as osp, \
           tc.tile_pool(name="accC", bufs=3, space="PSUM") as accc:
          wo_sb = []
          for k in range(KT_TILES):
              w = wp.tile([128, C], fr, name="w", tag="w")
              nc.sync.dma_start(out=w[:], in_=wo[ts(k, 128), :])
              wo_sb.append(w)
          for m in range(KT_TILES):
              acc = accc.tile([128, QBLK], f32, name="acc", tag="accC")
              for k in range(KT_TILES):
                  nc.tensor.matmul(acc[:], wo_sb[k][:, ts(m, 128)],
                                   att_sb[k][:],
                                   start=(k == 0), stop=(k == KT_TILES - 1))
              ost = osp.tile([128, QBLK], f32, name="ost", tag="ost")
              nc.scalar.activation(ost[:], acc[:], IDENT,
                                   bias=bo_sb[:, m:m + 1], scale=1.0)
              if small_out:
                  if m == 0:
                      nc.sync.dma_start(out=outT[:, :], in_=ost[:])
              else:
                  nc.sync.dma_start(out=outT[ts(m, 128), :], in_=ost[:])


# ---------------------------------------------------------------------------
# Host-side: runner (one-time jit) + kernel() entry point
# ---------------------------------------------------------------------------

class _SpmdRunner:
    def __init__(self, nc, n_cores=NCORES):
        import jax
        import numpy as _np
        from jax.sharding import Mesh, PartitionSpec
        from jax.experimental.shard_map import shard_map
        import concourse.mybir as mybir
        from concourse import bass2jax
        from concourse.bass2jax import _bass_exec_p, install_neuronx_cc_hook

        install_neuronx_cc_hook()
        self.jax = jax
        self.n_cores = n_cores
        partition_name = (nc.partition_id_tensor.name
                          if nc.partition_id_tensor else None)
        in_names, out_names, out_avals, zero_outs = [], [], [], []
        for alloc in nc.m.functions[0].allocations:
            if not isinstance(alloc, mybir.MemoryLocationSet):
                continue
            name = alloc.memorylocations[0].name
            if alloc.kind == "ExternalInput":
                if name != partition_name:
                    in_names.append(name)
            elif alloc.kind == "ExternalOutput":
                out_names.append(name)
                shape = tuple(alloc.tensor_shape)
                dtype = mybir.dt.np(alloc.dtype)
                out_avals.append(jax.core.ShapedArray(shape, dtype))
                zero_outs.append(_np.zeros(shape, dtype))
        self.in_names, self.out_names = in_names, out_names
        self.out_avals, self.zero_outs = out_avals, zero_outs
        n_params, n_outs = len(in_names), len(out_names)
        all_in = list(in_names) + list(out_names)
        if partition_name is not None:
            all_in.append(partition_name)
        donate = tuple(range(n_params, n_params + n_outs))

        def _body(*args):
            operands = list(args)
            if partition_name is not None:
                operands.append(bass2jax.partition_id_tensor())
            outs = _bass_exec_p.bind(
                *operands, out_avals=tuple(out_avals),
                in_names=tuple(all_in), out_names=tuple(out_names),
                lowering_input_output_aliases=(),
                sim_require_finite=True, sim_require_nnan=True, nc=nc)
            return tuple(outs)

        devices = jax.devices()[:n_cores]
        self.mesh = Mesh(_np.asarray(devices), ("core",))
        in_specs = (PartitionSpec("core"),) * (n_params + n_outs)
        out_specs = (PartitionSpec("core"),) * n_outs
        self.sharded = jax.jit(
            shard_map(_body, mesh=self.mesh, in_specs=in_specs,
                      out_specs=out_specs, check_rep=False),
            donate_argnums=donate, keep_unused=True)
        self._PartitionSpec = PartitionSpec

    def set_inputs(self, in_maps):
        import jax
        from jax.sharding import NamedSharding
        per_core = [[np.asarray(m[name]) for name in self.in_names]
                    for m in in_maps]
        sharding = NamedSharding(self.mesh, self._PartitionSpec("core"))
        self._in = [
            jax.device_put(np.concatenate(
                [per_core[c][i] for c in range(self.n_cores)], axis=0),
                sharding)
            for i in range(len(self.in_names))
        ]
        jax.block_until_ready(self._in)

    def run(self):
        import jax
        zeros = [np.zeros((self.n_cores * z.shape[0], *z.shape[1:]), z.dtype)
                 for z in self.zero_outs]
        out = self.sharded(*self._in, *zeros)
        jax.block_until_ready(out)
        return out

    def results(self, out_arrs):
        return [
            {name: np.asarray(out_arrs[i]).reshape(
                self.n_cores, *self.out_avals[i].shape)[c]
             for i, name in enumerate(self.out_names)}
            for c in range(self.n_cores)
        ]


def _get_runner(nrep: int = 1):
    key = ("runner", nrep, os.environ.get("MHA_KDTYPE", "f32r"))
    if key not in _CACHE:
        nc = _build_nc(nrep=nrep, kdtype=os.environ.get("MHA_KDTYPE", "f32r"))
        _CACHE[key] = _SpmdRunner(nc)
    return _CACHE[key]


def _make_in_maps(x, Wq, bq, Wk, bk, Wv, bv, Wo, bo):
    wq = np.ascontiguousarray(Wq, np.float32)
    wk = np.ascontiguousarray(Wk, np.float32)
    wv = np.ascontiguousarray(Wv, np.float32)
    wo = np.ascontiguousarray(Wo, np.float32)
    bqh = np.ascontiguousarray(np.asarray(bq, np.float32).reshape(8, 128).T)
    bkh = np.ascontiguousarray(np.asarray(bk, np.float32).reshape(8, 128).T)
    boh = np.ascontiguousarray(np.asarray(bo, np.float32).reshape(8, 128).T)
    bvh = np.ascontiguousarray(np.asarray(bv, np.float32).reshape(1, C))
    in_maps = []
    for c in range(NCORES):
        b, g = c // GROUPS, c % GROUPS
        q0 = g * QBLK
        xb = np.asarray(x[b], np.float32)
        xrot = np.concatenate([xb[q0:], xb[:q0]], axis=0)
        xTc = np.ascontiguousarray(xrot.T)
        in_maps.append({"xT": xTc, "wq": wq, "wk": wk, "wv": wv, "wo": wo,
                        "bq": bqh, "bk": bkh, "bo": boh, "bv": bvh})
    return in_maps


def kernel(x, Wq, bq, Wk, bk, Wv, bv, Wo, bo):
    runner = _get_runner()
    runner.set_inputs(_make_in_maps(x, Wq, bq, Wk, bk, Wv, bv, Wo, bo))
    res = runner.results(runner.run())
    out = np.empty((B, N, C), np.float32)
    for c in range(NCORES):
        b, g = c // GROUPS, c % GROUPS
        out[b, g * QBLK:(g + 1) * QBLK, :] = res[c]["outT"].T
    return out
